# revision 10
# baseline (speedup 1.0000x reference)
"""Trainium2 Bass kernel for nn_CFTModule (channel-attention over pooled tokens).

Reference computation per batch b (x: [H=256, W=256, C=64] fp32):
  pooled[c, i, j] = mean of x[:, :, c] over 64x64 spatial block (i, j)   # [64, 4, 4]
  tokens = pooled.reshape(64, 16)
  qk = tokens @ w_qkv.T          # [64, 32];  q, k = qk[:, :16], qk[:, 16:]
  dots = q @ k.T * 0.25          # [64, 64]
  attn = softmax(dots, axis=-1)  # [64, 64]
  out[h, w, t] = gelu_exact(sum_c x[h, w, c] * attn[t, c])

Sharding: data-parallel over batch. 16 batches -> 8 cores x 2 batches.

Per-core kernel layout trick: view x row h as [128 pairs, 128 (l, c)] where
pixel w = 2p + l.  PE-transpose each row tile -> XT [(l, c), p] stored in
SBUF as fp16.  Pooling is folded into PE matmuls against a 0/1 indicator,
accumulated in PSUM.  Phase 2 computes out = XT.T @ A2 where A2 is the
block-diagonal [[attn.T, 0], [0, attn.T]] (fp16), giving [p, (l, t)] tiles
that DMA back contiguously.  GELU (exact/erf) on the scalar engine.
"""
import numpy as np
from contextlib import ExitStack

import concourse.bass as bass
import concourse.bacc as bacc
import concourse.tile as tile
from concourse import mybir
from concourse.bass_utils import run_bass_kernel_spmd

F32 = mybir.dt.float32
F16 = mybir.dt.float16
AF = mybir.ActivationFunctionType

N_CORES = 8
B_LOC = 2          # batches per core
H = 256            # image rows
DIM = 16
SCALE = DIM ** -0.5
POOL_SCALE = 1.0 / (64.0 * 64.0)

_CACHE = {}


def _build(reps=1, loop_n=1):
    nc = bacc.Bacc()
    x = nc.dram_tensor("x", [B_LOC, H, 128, 128], F32, kind="ExternalInput")
    w = nc.dram_tensor("w_qkv", [32, 16], F32, kind="ExternalInput")
    out = nc.dram_tensor("out", [B_LOC, H, 128, 128], F32, kind="ExternalOutput")

    ident_d = nc.inline_tensor(np.eye(128, dtype=np.float32), name="identc")
    ind_np = np.zeros((128, 4), np.float32)
    for p in range(128):
        ind_np[p, p // 32] = 1.0
    ind_d = nc.inline_tensor(ind_np, name="ind4c")

    with ExitStack() as ctx:
        tc = ctx.enter_context(tile.TileContext(nc))
        const = ctx.enter_context(tc.tile_pool(name="const", bufs=1))
        in_pool = ctx.enter_context(tc.tile_pool(name="xin", bufs=8))
        xt_store = ctx.enter_context(tc.tile_pool(name="xt", bufs=2))
        out_pool = ctx.enter_context(tc.tile_pool(name="og", bufs=3))
        attn_pool = ctx.enter_context(tc.tile_pool(name="attn", bufs=2))
        tp_pool = ctx.enter_context(tc.tile_pool(name="tp", bufs=2, space="PSUM"))
        op_pool = ctx.enter_context(tc.tile_pool(name="op", bufs=2, space="PSUM"))
        pacc_pool = ctx.enter_context(tc.tile_pool(name="pacc", bufs=1, space="PSUM"))
        sm_pool = ctx.enter_context(tc.tile_pool(name="sm", bufs=2, space="PSUM"))
        a2_pool = ctx.enter_context(tc.tile_pool(name="a2", bufs=1, space="PSUM"))

        ident = const.tile([128, 128], F32, tag="ident")
        nc.sync.dma_start(ident[:], ident_d[:])
        ind4 = const.tile([128, 4], F32, tag="ind4")
        nc.sync.dma_start(ind4[:], ind_d[:])
        w_sb = const.tile([32, 16], F32, tag="w_sb")
        nc.sync.dma_start(w_sb[:], w[:])
        # wT[n, d] = w[d, n]
        wT_ps = sm_pool.tile([16, 32], F32, tag="sm")
        nc.tensor.matmul(wT_ps[:], lhsT=w_sb[:], rhs=ident[0:32, 0:32],
                         is_transpose=True)
        wT_sb = const.tile([16, 32], F32, tag="wT")
        nc.scalar.copy(wT_sb[:], wT_ps[:])

        def pass1_group(b, g, xt_all, pacc):
            """Rows 4g..4g+3: DMA in, PE transpose into one PSUM bank,
            pooling matmuls, then one fp32->fp16 copy into the XT store."""
            tp = tp_pool.tile([128, 512], F32, tag="tp")
            for q in range(4):
                h = 4 * g + q
                xt = in_pool.tile([128, 128], F32, tag="xin")
                nc.sync.dma_start(xt[:], x[b, h])
                nc.tensor.matmul(tp[:, q * 128:(q + 1) * 128], lhsT=xt[:],
                                 rhs=ident[:], is_transpose=True)
                i = h // 64
                nc.tensor.matmul(pacc[:, i * 4:(i + 1) * 4], lhsT=xt[:, 0:64],
                                 rhs=ind4[:], start=(h % 64 == 0), stop=False)
                nc.tensor.matmul(pacc[:, i * 4:(i + 1) * 4], lhsT=xt[:, 64:128],
                                 rhs=ind4[:], start=False, stop=(h % 64 == 63))
            nc.vector.tensor_copy(xt_all[:, g * 512:(g + 1) * 512], tp[:])

        def attn_block(pacc):
            """pooled sums [64, 16] -> attn -> A2 (block-diag attn.T, fp16)."""
            pooled_sb = attn_pool.tile([64, 16], F32, tag="pooled")
            nc.scalar.mul(pooled_sb[:], pacc[:], POOL_SCALE)
            ptT_ps = sm_pool.tile([16, 64], F32, tag="sm")
            nc.tensor.matmul(ptT_ps[:], lhsT=pooled_sb[:], rhs=ident[0:64, 0:64],
                             is_transpose=True)
            ptT_sb = attn_pool.tile([16, 64], F32, tag="ptT")
            nc.scalar.copy(ptT_sb[:], ptT_ps[:])
            qT_ps = sm_pool.tile([16, 64], F32, tag="sm")
            nc.tensor.matmul(qT_ps[:], lhsT=wT_sb[:, 0:16], rhs=ptT_sb[:])
            qT_sb = attn_pool.tile([16, 64], F32, tag="qT")
            nc.scalar.copy(qT_sb[:], qT_ps[:])
            kT_ps = sm_pool.tile([16, 64], F32, tag="sm")
            nc.tensor.matmul(kT_ps[:], lhsT=wT_sb[:, 16:32], rhs=ptT_sb[:])
            kT_sb = attn_pool.tile([16, 64], F32, tag="kT")
            nc.scalar.copy(kT_sb[:], kT_ps[:])
            dots_ps = sm_pool.tile([64, 64], F32, tag="sm")
            nc.tensor.matmul(dots_ps[:], lhsT=qT_sb[:], rhs=kT_sb[:])
            m = attn_pool.tile([64, 1], F32, tag="m")
            nc.vector.reduce_max(m[:], dots_ps[:], axis=mybir.AxisListType.X)
            negm = attn_pool.tile([64, 1], F32, tag="negm")
            nc.vector.tensor_scalar_mul(negm[:], m[:], -SCALE)
            e_sb = attn_pool.tile([64, 64], F32, tag="e")
            nc.scalar.activation(e_sb[:], dots_ps[:], AF.Exp, bias=negm[:],
                                 scale=SCALE)
            s = attn_pool.tile([64, 1], F32, tag="s")
            nc.vector.reduce_sum(s[:], e_sb[:], axis=mybir.AxisListType.X)
            r = attn_pool.tile([64, 1], F32, tag="r")
            nc.vector.reciprocal(r[:], s[:])
            attn_sb = attn_pool.tile([64, 64], F32, tag="attn")
            nc.vector.tensor_scalar_mul(attn_sb[:], e_sb[:], r[:])
            A2_ps = a2_pool.tile([128, 128], F32, tag="A2ps")
            nc.vector.memset(A2_ps[:], 0.0)
            # attnT via regular matmul (attn.T @ I); transpose-mode outputs
            # must start at PSUM partition 0, regular matmuls can col-tile.
            nc.tensor.matmul(A2_ps[0:64, 0:64], lhsT=attn_sb[:],
                             rhs=ident[0:64, 0:64])
            nc.tensor.matmul(A2_ps[64:128, 64:128], lhsT=attn_sb[:],
                             rhs=ident[0:64, 0:64])
            A2_sb = attn_pool.tile([128, 128], F16, tag="A2")
            nc.vector.tensor_copy(A2_sb[:], A2_ps[:])
            return A2_sb

        def phase2_group(b, g, xt_all, A2_sb):
            """Rows 4g..4g+3: out = XT.T @ A2 per row into one PSUM bank,
            one GELU over the bank, one DMA out."""
            op = op_pool.tile([128, 512], F32, tag="op")
            for q in range(4):
                h = 4 * g + q
                nc.tensor.matmul(op[:, q * 128:(q + 1) * 128],
                                 lhsT=xt_all[:, h * 128:(h + 1) * 128],
                                 rhs=A2_sb[:])
            og = out_pool.tile([128, 512], F32, tag="og")
            nc.scalar.activation(og[:], op[:], AF.Gelu)
            dram_view = out[b, 4 * g:4 * g + 4].transpose([1, 0, 2])
            nc.sync.dma_start(dram_view, og[:].rearrange("p (q f) -> p q f", q=4))

        def whole_body():
            # batch 0 pass 1
            xt0 = xt_store.tile([128, 64 * 512], F16, tag="xtall")
            pacc0 = pacc_pool.tile([64, 16], F32, tag="pacc")
            for g in range(64):
                pass1_group(0, g, xt0, pacc0)
            A2_0 = attn_block(pacc0)
            # batch 0 pass 2 interleaved with batch 1 pass 1
            xt1 = xt_store.tile([128, 64 * 512], F16, tag="xtall")
            pacc1 = pacc_pool.tile([64, 16], F32, tag="pacc")
            for g in range(64):
                phase2_group(0, g, xt0, A2_0)
                pass1_group(1, g, xt1, pacc1)
            A2_1 = attn_block(pacc1)
            for g in range(64):
                phase2_group(1, g, xt1, A2_1)

        if loop_n > 1:
            with tc.For_i(0, loop_n, 1):
                whole_body()
        else:
            for _ in range(reps):
                whole_body()

    nc.compile()
    return nc


def _get_nc(reps=1, loop_n=1):
    key = ("nc", reps, loop_n)
    if key not in _CACHE:
        _CACHE[key] = _build(reps, loop_n)
    return _CACHE[key]


def kernel(x, w_qkv):
    x = np.ascontiguousarray(np.asarray(x, dtype=np.float32))
    w_qkv = np.ascontiguousarray(np.asarray(w_qkv, dtype=np.float32))
    B, Hh, Ww, C = x.shape
    assert (B, Hh, Ww, C) == (16, 256, 256, 64), x.shape
    nc = _get_nc()
    xs = x.reshape(N_CORES, B_LOC, H, 128, 128)
    in_maps = [{"x": np.ascontiguousarray(xs[i]), "w_qkv": w_qkv}
               for i in range(N_CORES)]
    res = run_bass_kernel_spmd(nc, in_maps, core_ids=list(range(N_CORES)))
    outs = np.stack([res.results[i]["out"] for i in range(N_CORES)])
    return outs.reshape(16, 256, 256, 64)


# revision 15
# speedup vs baseline: 2.0728x; 2.0728x over previous
"""Trainium2 Bass kernel for nn_CFTModule (channel-attention over pooled tokens).

Reference computation per batch b (x: [H=256, W=256, C=64] fp32):
  pooled[c, i, j] = mean of x[:, :, c] over 64x64 spatial block (i, j)   # [64, 4, 4]
  tokens = pooled.reshape(64, 16)
  qk = tokens @ w_qkv.T          # [64, 32];  q, k = qk[:, :16], qk[:, 16:]
  dots = q @ k.T * 0.25          # [64, 64]
  attn = softmax(dots, axis=-1)  # [64, 64]
  out[h, w, t] = gelu_exact(sum_c x[h, w, c] * attn[t, c])

Sharding: data-parallel over batch. 16 batches -> 8 cores x 2 batches.

Per-core kernel layout trick: view x row h as [128 pairs, 128 (l, c)] where
pixel w = 2p + l.  PE-transpose each row tile -> XT [(l, c), p] stored in
SBUF as fp16.  Pooling is folded into PE matmuls against a 0/1 indicator,
accumulated in PSUM.  Phase 2 computes out = XT.T @ A2 where A2 is the
block-diagonal [[attn.T, 0], [0, attn.T]] (fp16), giving [p, (l, t)] tiles
that DMA back contiguously.  GELU (exact/erf) on the scalar engine.
"""
import numpy as np
from contextlib import ExitStack

import concourse.bass as bass
import concourse.bacc as bacc
import concourse.tile as tile
from concourse import mybir
from concourse.bass_utils import run_bass_kernel_spmd

F32 = mybir.dt.float32
F16 = mybir.dt.float16
AF = mybir.ActivationFunctionType

N_CORES = 8
B_LOC = 2          # batches per core
H = 256            # image rows
DIM = 16
SCALE = DIM ** -0.5
POOL_SCALE = 1.0 / (64.0 * 64.0)

_CACHE = {}


def _build(reps=1, loop_n=1):
    nc = bacc.Bacc()
    x = nc.dram_tensor("x", [B_LOC, H, 128, 128], F32, kind="ExternalInput")
    w = nc.dram_tensor("w_qkv", [32, 16], F32, kind="ExternalInput")
    out = nc.dram_tensor("out", [B_LOC, H, 128, 128], F32, kind="ExternalOutput")

    ident_d = nc.inline_tensor(np.eye(128, dtype=np.float32), name="identc")
    ind_np = np.zeros((128, 4), np.float32)
    for p in range(128):
        ind_np[p, p // 32] = 1.0
    ind_d = nc.inline_tensor(ind_np, name="ind4c")
    fold_np = np.vstack([np.eye(64, dtype=np.float32),
                         np.eye(64, dtype=np.float32)])
    fold_d = nc.inline_tensor(fold_np, name="foldc")

    with ExitStack() as ctx:
        tc = ctx.enter_context(tile.TileContext(nc))
        const = ctx.enter_context(tc.tile_pool(name="const", bufs=1))
        in_pool = ctx.enter_context(tc.tile_pool(name="xin", bufs=4))
        xt_store = ctx.enter_context(tc.tile_pool(name="xt", bufs=2))
        out_pool = ctx.enter_context(tc.tile_pool(name="og", bufs=3))
        attn_pool = ctx.enter_context(tc.tile_pool(name="attn", bufs=2))
        tp_pool = ctx.enter_context(tc.tile_pool(name="tp", bufs=2, space="PSUM"))
        op_pool = ctx.enter_context(tc.tile_pool(name="op", bufs=2, space="PSUM"))
        pacc_pool = ctx.enter_context(tc.tile_pool(name="pacc", bufs=1, space="PSUM"))
        sm_pool = ctx.enter_context(tc.tile_pool(name="sm", bufs=2, space="PSUM"))
        a2_pool = ctx.enter_context(tc.tile_pool(name="a2", bufs=1, space="PSUM"))

        ident = const.tile([128, 128], F32, tag="ident")
        nc.sync.dma_start(ident[:], ident_d[:])
        ind4 = const.tile([128, 4], F32, tag="ind4")
        nc.sync.dma_start(ind4[:], ind_d[:])
        fold = const.tile([128, 64], F32, tag="fold")
        nc.sync.dma_start(fold[:], fold_d[:])
        w_sb = const.tile([32, 16], F32, tag="w_sb")
        nc.sync.dma_start(w_sb[:], w[:])
        # wT[n, d] = w[d, n]
        wT_ps = sm_pool.tile([16, 32], F32, tag="sm")
        nc.tensor.matmul(wT_ps[:], lhsT=w_sb[:], rhs=ident[0:32, 0:32],
                         is_transpose=True)
        wT_sb = const.tile([16, 32], F32, tag="wT")
        # fold the pooling 1/4096 mean into the qkv weight (linear pass-through)
        nc.vector.tensor_scalar_mul(wT_sb[:], wT_ps[:], POOL_SCALE)

        def pass1_super(b, m, xt_all, pacc):
            """Rows 16m..16m+15: one 1 MiB DMA, then per 4-row group: PE
            transposes into a PSUM bank + per-row pooling matmul, and one
            fp32->fp16 copy into the XT store."""
            xt16 = in_pool.tile([128, 16 * 128], F32, tag="xin")
            nc.sync.dma_start(
                xt16[:].rearrange("p (r f) -> p r f", r=16),
                x[b, 16 * m:16 * m + 16].transpose([1, 0, 2]))
            for gq in range(4):
                g = 4 * m + gq
                tp = tp_pool.tile([128, 512], F32, tag="tp")
                for q in range(4):
                    h = 4 * g + q
                    r = h - 16 * m
                    xt = xt16[:, r * 128:(r + 1) * 128]
                    nc.tensor.matmul(tp[:, q * 128:(q + 1) * 128], lhsT=xt,
                                     rhs=ident[:], is_transpose=True)
                    i = h // 64
                    nc.tensor.matmul(pacc[:, i * 4:(i + 1) * 4], lhsT=xt,
                                     rhs=ind4[:], start=(h % 64 == 0),
                                     stop=(h % 64 == 63))
                nc.vector.tensor_copy(xt_all[:, g * 512:(g + 1) * 512], tp[:])

        def attn_block(pacc):
            """pooled sums [64, 16] -> attn -> A2 (block-diag attn.T, fp16)."""
            pacc_sb = attn_pool.tile([128, 16], F32, tag="pacc_sb")
            nc.vector.tensor_copy(pacc_sb[:], pacc[:])
            # one matmul: pacc_sb.T @ [I;I] = even/odd fold + transpose
            ptT_ps = sm_pool.tile([16, 64], F32, tag="sm")
            nc.tensor.matmul(ptT_ps[:], lhsT=pacc_sb[:], rhs=fold[:])
            ptT_sb = attn_pool.tile([16, 64], F32, tag="ptT")
            nc.vector.tensor_copy(ptT_sb[:], ptT_ps[:])
            qT_ps = sm_pool.tile([16, 64], F32, tag="sm")
            nc.tensor.matmul(qT_ps[:], lhsT=wT_sb[:, 0:16], rhs=ptT_sb[:])
            qT_sb = attn_pool.tile([16, 64], F32, tag="qT")
            nc.vector.tensor_copy(qT_sb[:], qT_ps[:])
            kT_ps = sm_pool.tile([16, 64], F32, tag="sm")
            nc.tensor.matmul(kT_ps[:], lhsT=wT_sb[:, 16:32], rhs=ptT_sb[:])
            kT_sb = attn_pool.tile([16, 64], F32, tag="kT")
            nc.vector.tensor_copy(kT_sb[:], kT_ps[:])
            dots_ps = sm_pool.tile([64, 64], F32, tag="sm")
            nc.tensor.matmul(dots_ps[:], lhsT=qT_sb[:], rhs=kT_sb[:])
            m = attn_pool.tile([64, 1], F32, tag="m")
            nc.vector.reduce_max(m[:], dots_ps[:], axis=mybir.AxisListType.X)
            negm = attn_pool.tile([64, 1], F32, tag="negm")
            nc.vector.tensor_scalar_mul(negm[:], m[:], -SCALE)
            e_sb = attn_pool.tile([64, 64], F32, tag="e")
            nc.scalar.activation(e_sb[:], dots_ps[:], AF.Exp, bias=negm[:],
                                 scale=SCALE)
            s = attn_pool.tile([64, 1], F32, tag="s")
            nc.vector.reduce_sum(s[:], e_sb[:], axis=mybir.AxisListType.X)
            r = attn_pool.tile([64, 1], F32, tag="r")
            nc.vector.reciprocal(r[:], s[:])
            attn_sb = attn_pool.tile([64, 64], F32, tag="attn")
            nc.vector.tensor_scalar_mul(attn_sb[:], e_sb[:], r[:])
            A2_ps = a2_pool.tile([128, 128], F32, tag="A2ps")
            nc.vector.memset(A2_ps[:], 0.0)
            # attnT via regular matmul (attn.T @ I); transpose-mode outputs
            # must start at PSUM partition 0, regular matmuls can col-tile.
            nc.tensor.matmul(A2_ps[0:64, 0:64], lhsT=attn_sb[:],
                             rhs=ident[0:64, 0:64])
            nc.tensor.matmul(A2_ps[64:128, 64:128], lhsT=attn_sb[:],
                             rhs=ident[0:64, 0:64])
            A2_sb = attn_pool.tile([128, 128], F16, tag="A2")
            nc.vector.tensor_copy(A2_sb[:], A2_ps[:])
            return A2_sb

        def phase2_super(b, m8, xt_all, A2_sb):
            """Rows 8*m8..8*m8+7: per 4 rows one PSUM bank of matmuls and one
            GELU into half of og; one DMA out per 8 rows."""
            og = out_pool.tile([128, 1024], F32, tag="og")
            for half in range(2):
                g = 2 * m8 + half
                op = op_pool.tile([128, 512], F32, tag="op")
                for q in range(4):
                    h = 4 * g + q
                    nc.tensor.matmul(op[:, q * 128:(q + 1) * 128],
                                     lhsT=xt_all[:, h * 128:(h + 1) * 128],
                                     rhs=A2_sb[:])
                nc.scalar.activation(og[:, half * 512:(half + 1) * 512], op[:],
                                     AF.Gelu)
            dram_view = out[b, 8 * m8:8 * m8 + 8].transpose([1, 0, 2])
            nc.gpsimd.dma_start(dram_view, og[:].rearrange("p (q f) -> p q f", q=8))

        def whole_body():
            # batch 0 pass 1
            xt0 = xt_store.tile([128, 64 * 512], F16, tag="xtall")
            pacc0 = pacc_pool.tile([128, 16], F32, tag="pacc")
            for m in range(16):
                pass1_super(0, m, xt0, pacc0)
            A2_0 = attn_block(pacc0)
            # batch 0 pass 2 interleaved with batch 1 pass 1
            xt1 = xt_store.tile([128, 64 * 512], F16, tag="xtall")
            pacc1 = pacc_pool.tile([128, 16], F32, tag="pacc")
            for m in range(16):
                phase2_super(0, 2 * m, xt0, A2_0)
                phase2_super(0, 2 * m + 1, xt0, A2_0)
                pass1_super(1, m, xt1, pacc1)
            A2_1 = attn_block(pacc1)
            for m in range(32):
                phase2_super(1, m, xt1, A2_1)

        if loop_n > 1:
            with tc.For_i(0, loop_n, 1):
                whole_body()
        else:
            for _ in range(reps):
                whole_body()

    nc.compile()
    return nc


def _get_nc(reps=1, loop_n=1):
    key = ("nc", reps, loop_n)
    if key not in _CACHE:
        _CACHE[key] = _build(reps, loop_n)
    return _CACHE[key]


def kernel(x, w_qkv):
    x = np.ascontiguousarray(np.asarray(x, dtype=np.float32))
    w_qkv = np.ascontiguousarray(np.asarray(w_qkv, dtype=np.float32))
    B, Hh, Ww, C = x.shape
    assert (B, Hh, Ww, C) == (16, 256, 256, 64), x.shape
    nc = _get_nc()
    xs = x.reshape(N_CORES, B_LOC, H, 128, 128)
    in_maps = [{"x": np.ascontiguousarray(xs[i]), "w_qkv": w_qkv}
               for i in range(N_CORES)]
    res = run_bass_kernel_spmd(nc, in_maps, core_ids=list(range(N_CORES)))
    outs = np.stack([res.results[i]["out"] for i in range(N_CORES)])
    return outs.reshape(16, 256, 256, 64)
